# revision 1
# baseline (speedup 1.0000x reference)
"""NCNPredictor v5: bf16 adjacency + scalar_tensor_tensor accumulate.

DVE chain per 128-edge tile (4 wide ops + 2 tiny combines):
  t_all = gi * gj                                  (TT, 3750 bf16)
  u_all = (t_all * 1) * Ycat,  aA = sum            (STT accum: terms 1+3+4)
  (u0 * -1) * t1,              aB = -sum(u0*t1)    (STT accum: term 2)
  (u2 * -1) * t0,              aC = -sum(u2*t0)    (STT accum: term 5)
  acc = aA + aB + aC                               (2 tiny TT adds)
Bias is added host-side in combine.
"""

import sys
from contextlib import ExitStack

import numpy as np

sys.path.insert(0, "/opt/trn_rl_repo")

import concourse.bass as bass
import concourse.tile as tile
from concourse import bacc, mybir
from concourse.bass_utils import run_bass_kernel_spmd

N = 10000
D = 128
E = 8192
NCORES = 8
NCOL = N // NCORES
W3 = 3 * NCOL
E_OWN = E // NCORES
P = 128
ET = E // P
ET_OWN = E_OWN // P
F32 = mybir.dt.float32
BF16 = mybir.dt.bfloat16
I32 = mybir.dt.int32
MUL = mybir.AluOpType.mult
ADD = mybir.AluOpType.add

_CACHE = {}


def _build_nc():
    nc = bacc.Bacc(num_swdge_queues=4)

    acat = nc.declare_dram_parameter("acat", [N, W3], BF16, False)
    xw = nc.declare_dram_parameter("xw", [N, D], F32, False)
    xr = nc.declare_dram_parameter("xr", [N, D], F32, False)
    ycat = nc.declare_dram_parameter("ycat", [P, W3], BF16, False)
    ii = nc.declare_dram_parameter("ii", [E, 1], I32, False)
    jj = nc.declare_dram_parameter("jj", [E, 1], I32, False)
    iown = nc.declare_dram_parameter("iown", [E_OWN, 1], I32, False)
    jown = nc.declare_dram_parameter("jown", [E_OWN, 1], I32, False)

    out_cn = nc.declare_dram_parameter("out_cn", [E, 1], F32, True)
    out_xij = nc.declare_dram_parameter("out_xij", [E_OWN, 1], F32, True)

    with tile.TileContext(nc) as tc, ExitStack() as ctx:
        const = ctx.enter_context(tc.tile_pool(name="const", bufs=1))
        yc = const.tile([P, W3], BF16)
        nc.sync.dma_start(yc[:], ycat[:])

        idxp = ctx.enter_context(tc.tile_pool(name="idxp", bufs=3))
        gat = ctx.enter_context(tc.tile_pool(name="gat", bufs=3))
        msk = ctx.enter_context(tc.tile_pool(name="msk", bufs=2))
        scr = ctx.enter_context(tc.tile_pool(name="scr", bufs=2))
        accp = ctx.enter_context(tc.tile_pool(name="accp", bufs=2))

        for et in range(ET):
            e0 = et * P
            ii_t = idxp.tile([P, 1], I32, name="ii_t")
            nc.sync.dma_start(ii_t[:], ii[e0 : e0 + P, :])
            jj_t = idxp.tile([P, 1], I32, name="jj_t")
            nc.sync.dma_start(jj_t[:], jj[e0 : e0 + P, :])

            gi = gat.tile([P, W3], BF16, name="gi")
            nc.gpsimd.indirect_dma_start(
                out=gi[:], out_offset=None, in_=acat[:],
                in_offset=bass.IndirectOffsetOnAxis(ap=ii_t[:, :1], axis=0),
            )
            gj = gat.tile([P, W3], BF16, name="gj")
            nc.gpsimd.indirect_dma_start(
                out=gj[:], out_offset=None, in_=acat[:],
                in_offset=bass.IndirectOffsetOnAxis(ap=jj_t[:, :1], axis=0),
            )

            t_all = msk.tile([P, W3], BF16, name="t_all")
            nc.vector.tensor_tensor(out=t_all[:], in0=gi[:], in1=gj[:], op=MUL)

            u_all = msk.tile([P, W3], BF16, name="u_all")
            oj = scr.tile([P, NCOL], BF16, name="oj")
            oj2 = scr.tile([P, NCOL], BF16, name="oj2")
            a = [accp.tile([P, 1], F32, name=f"a{s}") for s in range(5)]
            nc.vector.scalar_tensor_tensor(
                out=u_all[:], in0=t_all[:], scalar=1.0, in1=yc[:],
                op0=MUL, op1=MUL, accum_out=a[0][:],
            )
            nc.vector.scalar_tensor_tensor(
                out=oj[:], in0=u_all[:, 0:NCOL], scalar=-1.0,
                in1=t_all[:, NCOL : 2 * NCOL], op0=MUL, op1=MUL,
                accum_out=a[1][:],
            )
            nc.vector.scalar_tensor_tensor(
                out=oj2[:], in0=u_all[:, 2 * NCOL : W3], scalar=-1.0,
                in1=t_all[:, 0:NCOL], op0=MUL, op1=MUL, accum_out=a[2][:],
            )
            nc.vector.tensor_tensor(out=a[3][:], in0=a[0][:], in1=a[1][:], op=ADD)
            nc.vector.tensor_tensor(out=a[4][:], in0=a[3][:], in1=a[2][:], op=ADD)
            nc.sync.dma_start(out_cn[e0 : e0 + P, :], a[4][:])

        for et in range(ET_OWN):
            e0 = et * P
            io_t = idxp.tile([P, 1], I32, name="io_t")
            nc.sync.dma_start(io_t[:], iown[e0 : e0 + P, :])
            jo_t = idxp.tile([P, 1], I32, name="jo_t")
            nc.sync.dma_start(jo_t[:], jown[e0 : e0 + P, :])

            xi_t = gat.tile([P, D], F32, name="xi_t")
            nc.gpsimd.indirect_dma_start(
                out=xi_t[:], out_offset=None, in_=xw[:],
                in_offset=bass.IndirectOffsetOnAxis(ap=io_t[:, :1], axis=0),
            )
            xj_t = gat.tile([P, D], F32, name="xj_t")
            nc.gpsimd.indirect_dma_start(
                out=xj_t[:], out_offset=None, in_=xr[:],
                in_offset=bass.IndirectOffsetOnAxis(ap=jo_t[:, :1], axis=0),
            )
            oxe = scr.tile([P, D], F32, name="oxe")
            oxa = accp.tile([P, 1], F32, name="oxa")
            nc.vector.scalar_tensor_tensor(
                out=oxe[:], in0=xi_t[:], scalar=1.0, in1=xj_t[:],
                op0=MUL, op1=MUL, accum_out=oxa[:],
            )
            nc.sync.dma_start(out_xij[e0 : e0 + P, :], oxa[:])

    return nc


def get_nc():
    if "nc" not in _CACHE:
        nc = _build_nc()
        nc.compile()
        _CACHE["nc"] = nc
    return _CACHE["nc"]


def make_in_maps(x, adj_0_1, adj_1, adj_0_1_2, tar_ei, Wxs, bxs):
    import ml_dtypes

    bf = ml_dtypes.bfloat16
    x32 = np.ascontiguousarray(x, dtype=np.float32)
    wxs = np.asarray(Wxs, dtype=np.float32)
    w0 = wxs[0:D, 0]
    wy = np.concatenate(
        [wxs[D : 2 * D], wxs[2 * D : 3 * D], wxs[3 * D : 4 * D]], axis=1
    )
    y = x32 @ wy
    xwf = np.ascontiguousarray(x32 * w0[None, :])
    ii_all = np.ascontiguousarray(tar_ei[0].astype(np.int32).reshape(E, 1))
    jj_all = np.ascontiguousarray(tar_ei[1].astype(np.int32).reshape(E, 1))

    a01b = adj_0_1.astype(bf)
    a1b = adj_1.astype(bf)
    a012b = adj_0_1_2.astype(bf)
    yb = y.astype(bf)

    in_maps = []
    for c in range(NCORES):
        c0 = c * NCOL
        esl = slice(c * E_OWN, (c + 1) * E_OWN)
        acat = np.empty((N, W3), dtype=bf)
        acat[:, 0:NCOL] = a01b[:, c0 : c0 + NCOL]
        acat[:, NCOL : 2 * NCOL] = a1b[:, c0 : c0 + NCOL]
        acat[:, 2 * NCOL : W3] = a012b[:, c0 : c0 + NCOL]
        ycat = np.empty((P, W3), dtype=bf)
        for k in range(3):
            ycat[:, k * NCOL : (k + 1) * NCOL] = yb[c0 : c0 + NCOL, k][None, :]
        in_maps.append({
            "acat": acat,
            "xw": xwf,
            "xr": x32,
            "ycat": ycat,
            "ii": ii_all,
            "jj": jj_all,
            "iown": np.ascontiguousarray(ii_all[esl]),
            "jown": np.ascontiguousarray(jj_all[esl]),
        })
    return in_maps


def combine_results(results, b):
    out = np.zeros((E, 1), dtype=np.float64)
    for c in range(NCORES):
        out += results[c]["out_cn"].astype(np.float64)
        out[c * E_OWN : (c + 1) * E_OWN] += results[c]["out_xij"].astype(np.float64)
    return (out + b).astype(np.float32)


def kernel(x, adj_0_1, adj_1, adj_0_1_2, tar_ei, Wxs, bxs):
    nc = get_nc()
    in_maps = make_in_maps(x, adj_0_1, adj_1, adj_0_1_2, tar_ei, Wxs, bxs)
    res = run_bass_kernel_spmd(nc, in_maps, list(range(NCORES)))
    b = float(np.asarray(bxs, dtype=np.float32).reshape(-1)[0])
    return combine_results(res.results, b)

